# revision 13
# baseline (speedup 1.0000x reference)
"""Trainium2 kernel for nn_Mixing: FFT-based causal conv (length-N linear
convolution along tokens) + LayerNorm + residual.

The reference computes, per (batch, channel):
    conv[t] = sum_{s<=t} x[s] * w[t-s]          (causal linear conv, N=4096)
then LayerNorm over D=1024 channels and a residual add.

The conv is a lower-triangular Toeplitz matmul. With 128-token blocks there
are only NT=32 distinct 128x128 blocks B_d[c, r] = w[128*d + r - c] (zero
where the index is negative), and

    out_tile[i] = sum_{j<=i} B_{i-j}^T @ x_tile[j]

which maps directly onto the TensorEngine (lhsT = B_d, rhs = x_tile, both
fp16, accumulated in PSUM fp32). The Toeplitz blocks are built on the host
from `weights` (cheap gather) and passed as an extra input; x is also cast
to fp16 on the host, halving the input DMA.  The block-level computation
uses a two-level Karatsuba (3-mult Toeplitz) decomposition: 384 block-MACs
instead of the naive 528.  Deeper splits do not pay: at the h<=8 level a
split trades 1 saved MAC for ~1 MAC-equivalent of fold/copy work.

Scheduling notes (from perfetto traces):
  - The PE matmul stream runs at peak cadence (~219ns per 512-row half);
    the only PE stalls are PSUM-pool recycling waits, i.e. a new matmul
    group waiting for the previous tile's LayerNorm chain to drain its
    PSUM banks.  So every PSUM accumulation is drained *immediately* per
    half (copy or first LN-add right after each half's stop).
  - Product prep work is spread across tiles by need-date instead of
    bursts at i=8/16/24.
  - The nrm/residual/DMA epilogue is software-pipelined one tile late so
    per-engine queues never head-of-line-block a PSUM-releasing copy.
  - The last tile folds its single LN partial into PSUM via an
    identity-block matmul, shortening the serial tail.

Sharding: data-parallel over batch B=8 across the 8 NeuronCores (one batch
per core, no communication).  Output is fp16 (upcast to fp32 on the host),
halving the output DMA.
"""

import numpy as np

B, N, D = 8, 4096, 1024
P = 128
NT = N // P  # 32 token tiles
HALF = 512  # PSUM bank = 512 fp32
LN_EPS = 1e-5
ID_SLOT = 31  # unused toep slot, holds the 128x128 identity

_CACHE: dict = {}


def _build_program():
    import concourse.bass as bass  # noqa: F401
    import concourse.tile as tile
    from concourse import bacc, mybir
    from concourse.tile import add_dep_helper

    f32 = mybir.dt.float32
    f16 = mybir.dt.float16
    ADD = mybir.AluOpType.add

    nc = bacc.Bacc()
    x_in = nc.declare_dram_parameter("x16", [N, D], f16, isOutput=False)
    tp_in = nc.declare_dram_parameter("toep", [P, NT * P], f16, isOutput=False)
    tp2_in = nc.declare_dram_parameter("toep2", [P, NT * P], f16, isOutput=False)
    tp4_in = nc.declare_dram_parameter("toep4", [P, 32 * P], f16, isOutput=False)
    tp5_in = nc.declare_dram_parameter("toep5", [P, NT * P], f16, isOutput=False)
    out_t = nc.declare_dram_parameter("out", [N, D], f16, isOutput=True)

    x_t = x_in[:].rearrange("(n p) d -> n p d", p=P)
    o_t = out_t[:].rearrange("(n p) d -> n p d", p=P)
    tp_t = tp_in[:].rearrange("p (n r) -> p n r", r=P)
    tp2_t = tp2_in[:].rearrange("p (n r) -> p n r", r=P)
    tp4_t = tp4_in[:].rearrange("p (n r) -> p n r", r=P)
    tp5_t = tp5_in[:].rearrange("p (n r) -> p n r", r=P)

    HALVES = ((0, HALF), (HALF, D))

    with tile.TileContext(nc) as tc:
        with (
            tc.tile_pool(name="wt", bufs=1) as wt_pool,
            tc.tile_pool(name="xb", bufs=NT) as xb_pool,
            tc.tile_pool(name="xs", bufs=24) as xs_pool,
            tc.tile_pool(name="p14", bufs=6) as p14_pool,
            tc.tile_pool(name="p1s", bufs=8) as p1s_pool,
            tc.tile_pool(name="pd", bufs=4) as pd_pool,
            tc.tile_pool(name="sum", bufs=5) as sum_pool,
            tc.tile_pool(name="nrm", bufs=3) as nrm_pool,
            tc.tile_pool(name="res", bufs=3) as res_pool,
            tc.tile_pool(name="st", bufs=8) as st_pool,
            tc.tile_pool(name="ps", bufs=4, space="PSUM") as ps_pool,
        ):
            eps = wt_pool.tile([P, 1], f32, tag="eps")
            nc.vector.memset(eps[:], LN_EPS)
            # Force the one-time scalar ACT_TABLE_LOAD (~1.3us) during the
            # warm-up window instead of inside tile 0's LN chain.
            warm_std = st_pool.tile([P, 1], f32, tag="std")
            nc.scalar.activation(
                warm_std[:], eps[:], mybir.ActivationFunctionType.Sqrt,
                bias=eps[:],
            )

            # HAM warm-up: dummy matmuls on zeroed scratch while the first
            # DMAs are in flight (PE clock ungates after ~3.4us of activity).
            warm_w = wt_pool.tile([P, HALF], f16, tag="warmw")
            nc.vector.memset(warm_w[:], 0.0)
            warm_ps = ps_pool.tile([P, D], f32, tag="ps")
            for _ in range(12):
                nc.tensor.matmul(
                    warm_ps[:, 0:HALF], warm_w[:, 0:P], warm_w[:],
                    start=True, stop=True,
                )

            # Toeplitz blocks arrive as f16 from the host; load in chunks,
            # interleaved with the first x tiles, so step 0 starts early.
            tpb = wt_pool.tile([P, NT, P], f16, tag="tpb")
            xb = []

            def load_x(i):
                xbi = xb_pool.tile([P, D], f16, tag="xb")
                nc.sync.dma_start(xbi[:], x_t[i])
                xb.append(xbi)
                return xbi

            # Upfront loads cover the first ~10 tiles; the rest drip in
            # one group per tile iteration so the sync DMA queue stays
            # shallow and output DMAs are never stuck behind a deep input
            # backlog (the full input stream is ~35us of DMA).
            tp4 = wt_pool.tile([P, 32, P], f16, tag="tp4")
            tpd = wt_pool.tile([P, NT, P], f16, tag="tpd")
            tp5 = wt_pool.tile([P, NT, P], f16, tag="tp5")
            load_x(0)
            nc.sync.dma_start(tpb[:, 0:2, :], tp_t[:, 0:2, :])
            load_x(1)
            nc.sync.dma_start(tpb[:, 2:4, :], tp_t[:, 2:4, :])
            load_x(2)
            load_x(3)
            nc.sync.dma_start(tpb[:, 4:8, :], tp_t[:, 4:8, :])
            load_x(4)
            load_x(5)
            nc.sync.dma_start(tpb[:, 8:16, :], tp_t[:, 8:16, :])
            load_x(6)
            load_x(7)
            nc.sync.dma_start(tp4[:, 0:16, :], tp4_t[:, 0:16, :])
            load_x(8)
            load_x(9)

            drip = [
                lambda: (load_x(10), load_x(11), load_x(12)),
                lambda: (load_x(13), load_x(14), load_x(15)),
                lambda: (nc.sync.dma_start(tp4[:, 16:24, :],
                                           tp4_t[:, 16:24, :]),
                         load_x(16)),
                lambda: (load_x(17), load_x(18)),
                lambda: nc.sync.dma_start(tpb[:, 16:32, :], tp_t[:, 16:32, :]),
                lambda: (load_x(19), load_x(20)),
                lambda: nc.sync.dma_start(tp4[:, 24:32, :], tp4_t[:, 24:32, :]),
                lambda: (load_x(21), load_x(22)),
                lambda: nc.sync.dma_start(tpd[:], tp2_t),
                lambda: (load_x(23), load_x(24)),
                lambda: nc.sync.dma_start(tp5[:], tp5_t),
                lambda: (load_x(25), load_x(26)),
                lambda: (load_x(27), load_x(28)),
                lambda: (load_x(29), load_x(30)),
                lambda: load_x(31),
            ]

            # Toeplitz (Winograd 3-mult) trick, two levels.
            #
            # Level 1, the 16x16 block cross term out[16:32] += CROSS@x[0:16]
            # with equal diagonal quadrants (C11 = C22 = B[16+p-q]):
            #   out[16+p] += P1_p + sum_q (B[8+p-q]  - B[16+p-q]) x[8+q]
            #   out[24+p] += P1_p + sum_q (B[24+p-q] - B[16+p-q]) x[q]
            #   P1_p       = sum_q B[16+p-q] (x[q] + x[8+q])
            # (tpd holds the +-8 difference blocks).
            #
            # Level 2, the 8x8 crosses inside each 16-triangle (outputs 8-15
            # from x[0:8], and outputs 24-31 from x[16:24], both with matrix
            # B[8+p'-q']), same identity with 4x4 quadrants and +-4
            # difference blocks (tp4: slot e in [1,15] = B_e - B_{e+4},
            # slot 16+e = B_e - B_{e-4}).
            #
            # Level 3: the level-1 P1 product and the two +-8 diff products
            # (Dlo @ x[8:16] for tiles 16-23, Dhi @ x[0:8] for tiles 24-31)
            # are themselves 8x8 Toeplitz products and get the same 3-mult
            # split (Q1/PDlo/PDhi, second-order diffs in tp5).
            xsum = [None] * 8   # xs_q = x[q] + x[8+q]
            xs4l = [None] * 4   # x[q'] + x[4+q']
            xs4m = [None] * 4   # x[8+q'] + x[12+q']
            xss = [None] * 4    # xsum[q] + xsum[4+q] = xs4l[q] + xs4m[q]
            xs4h = [None] * 4   # x[16+q'] + x[20+q']
            xs4l2 = [None] * 4  # x[q'] + x[4+q'] (recomputed, xs4l recycled)
            p1sb = [None] * 8   # level-1 P1_p
            q1 = [None] * 4     # level-2 P1 inside the level-1 P1
            p1lo = [None] * 4   # level-2 P1 for tiles 8-15
            p1hi = [None] * 4   # level-2 P1 for tiles 24-31
            pdlo = [None] * 4   # P1 of the Dlo diff product (tiles 16-23)
            pdhi = [None] * 4   # P1 of the Dhi diff product
            pcmb = [None] * 4   # p1hi + PDhi combined (tiles 24-31)
            pfin = [None]       # last tile's single combined add

            def mm_half(pst, lhsT, rhs_tile, h, start, stop):
                lo, hi = HALVES[h]
                return nc.tensor.matmul(
                    pst[:, lo:hi], lhsT, rhs_tile[:, lo:hi],
                    start=start, stop=stop,
                )

            def tile_mm_pairs(i):
                # (lhsT AP, rhs tile) pairs accumulating out-tile i. Diff
                # MACs first (inputs long ready), triangle last (j=i arrives
                # latest).
                pairs = []
                if i < 8:
                    tri0 = 0
                elif i < 16:
                    pq = (i - 8) % 4
                    if i < 12:  # D4lo, e = 4+pq-q' in [1,7]
                        pairs += [(tp4[:, 4 + pq - q, :], xb[4 + q])
                                  for q in range(4)]
                    else:       # D4hi, e = 12+pq-q' in [9,15] -> slot 16+e
                        pairs += [(tp4[:, 19 + pq - q, :], xb[q])
                                  for q in range(4)]
                    tri0 = 8
                else:
                    p = i - 16
                    pq = p % 4
                    if p < 4:       # G_e = Dlo_e - Dlo_{e+4}, e in [1,7]
                        pairs += [(tp5[:, 4 + pq - q, :], xb[12 + q])
                                  for q in range(4)]
                    elif p < 8:     # H_e = Dlo_e - Dlo_{e-4}, e in [9,15]
                        pairs += [(tp5[:, 12 + pq - q, :], xb[8 + q])
                                  for q in range(4)]
                    elif p < 12:    # G2_e = Dhi_e - Dhi_{e+4}, e in [17,23]
                        pairs += [(tp5[:, 20 + pq - q, :], xb[4 + q])
                                  for q in range(4)]
                    else:           # H2_e = Dhi_e - Dhi_{e-4}, e in [25,31]
                        pairs += [(tp5[:, 28 + pq - q, :], xb[q])
                                  for q in range(4)]
                    if i < 24:
                        tri0 = 16
                    else:
                        pq = (i - 24) % 4
                        if i < 28:
                            pairs += [(tp4[:, 4 + pq - q, :], xb[20 + q])
                                      for q in range(4)]
                        else:
                            pairs += [(tp4[:, 19 + pq - q, :], xb[16 + q])
                                      for q in range(4)]
                        tri0 = 24
                pairs += [(tpb[:, i - j, :], xb[j]) for j in range(tri0, i + 1)]
                return pairs

            def tile_mms_half(i, ps, h, stop=True):
                pairs = tile_mm_pairs(i)
                n = len(pairs)
                inst = None
                for k, (lh, rh) in enumerate(pairs):
                    inst = mm_half(ps, lh, rh, h, k == 0,
                                   stop and k == n - 1)
                return inst

            def xsum_tile(a, b):
                xs = xs_pool.tile([P, D], f16, tag="xs")
                nc.vector.tensor_tensor(xs[:], a[:], b[:], op=ADD)
                return xs

            def product(terms, pool, tag):
                # sum_k lhsT_k @ rhs_k in PSUM, then one full-D scalar copy
                # to fp16 (PSUM-reading ops are latency-dominated, so one
                # full-D op beats two per-half ones).
                psp = ps_pool.tile([P, D], f32, tag="ps")
                out = pool.tile([P, D], f16, tag=tag)
                n = len(terms)
                for h in (0, 1):
                    for k, (lh, rh) in enumerate(terms):
                        mm_half(psp, lh, rh, h, k == 0, k == n - 1)
                nc.scalar.copy(out[:], psp[:])
                return out

            def product_plus(terms, addt, pool, tag):
                # Like product(), but the fp16 result is psum + addt
                # (one full-D vector TT from PSUM).
                psp = ps_pool.tile([P, D], f32, tag="ps")
                out = pool.tile([P, D], f16, tag=tag)
                n = len(terms)
                for h in (0, 1):
                    for k, (lh, rh) in enumerate(terms):
                        mm_half(psp, lh, rh, h, k == 0, k == n - 1)
                nc.vector.tensor_tensor(out[:], psp[:], addt[:], op=ADD)
                return out

            def preps_for(i):
                # PE/scalar prep work (Karatsuba P1 products) emitted at the
                # earliest tile where all inputs are ready and it is still
                # ahead of its first consumer.  Vector x-pair sums live in
                # xsums_for() at tile END so they never delay the
                # PSUM-draining adds at tile start.
                if 8 <= i < 12:
                    p = i - 8
                    p1lo[p] = product(
                        [(tpb[:, 8 + p - q, :], xs4l[q]) for q in range(4)],
                        p14_pool, "p14",
                    )
                if 16 <= i < 20:
                    p = i - 16
                    q1[p] = product(
                        [(tpb[:, 16 + p - q, :], xss[q]) for q in range(4)],
                        p14_pool, "p14",
                    )
                    p1sb[p] = product_plus(
                        [(tp4[:, 12 + p - q, :], xsum[4 + q]) for q in range(4)],
                        q1[p], p1s_pool, "p1s",
                    )
                    pdlo[p] = product(
                        [(tpd[:, 8 + p - q, :], xs4m[q]) for q in range(4)],
                        pd_pool, "pd",
                    )
                if 20 <= i < 24:
                    p = i - 20
                    p1sb[4 + p] = product_plus(
                        [(tp4[:, 27 + p - q, :], xsum[q]) for q in range(4)],
                        q1[p], p1s_pool, "p1s",
                    )
                if 21 <= i < 25:
                    p = i - 21
                    pdhi[p] = product(
                        [(tpd[:, 24 + p - q, :], xs4l2[q]) for q in range(4)],
                        p14_pool, "p14",
                    )
                if 23 <= i < 27:
                    p = i - 23
                    p1hi[p] = product(
                        [(tpb[:, 8 + p - q, :], xs4h[q]) for q in range(4)],
                        p14_pool, "p14",
                    )

            def xsums_for(i):
                # Vector x-pair sums, emitted at tile END one tile ahead of
                # the earliest consumer (x DMA runs far ahead of the PE).
                if 3 <= i < 7:
                    xs4l[i - 3] = xsum_tile(xb[i - 3], xb[i + 1])
                if 7 <= i < 15:
                    xsum[i - 7] = xsum_tile(xb[i - 7], xb[i + 1])
                if 11 <= i < 15:
                    xs4m[i - 11] = xsum_tile(xb[i - 3], xb[i + 1])
                if 12 <= i < 16:
                    q = i - 12
                    xss[q] = xsum_tile(xs4l[q], xs4m[q])
                if 17 <= i < 21:
                    q = i - 17
                    xs4l2[q] = xsum_tile(xb[q], xb[4 + q])
                if 19 <= i < 23:
                    p = i - 19
                    xs4h[p] = xsum_tile(xb[i - 3], xb[i + 1])
                if 23 <= i < 27:
                    p = i - 23
                    pcmb[p] = xsum_tile(p1hi[p], pdhi[p])
                if i == 27:
                    pfin[0] = xsum_tile(p1sb[7], pcmb[3])

            def ln_adds(i):
                # fp16 P1 tiles to add to the PSUM before LayerNorm.
                if i < 8:
                    return []
                if i < 16:
                    return [p1lo[(i - 8) % 4]]
                if i < 24:
                    return [p1sb[i - 16], pdlo[(i - 16) % 4]]
                if i == NT - 1:
                    return [pfin[0]]  # folded into PSUM via identity matmul
                return [p1sb[(i - 16) % 8], pcmb[(i - 24) % 4]]

            # Software-pipelined epilogue: tile i's sqrt/recip/nb/nrm/res/DMA
            # are emitted during tile i+1's body so they never
            # head-of-line-block a PSUM-releasing copy or add on the
            # scalar/vector queues.
            pending = []

            def flush_epilogue():
                while pending:
                    j, ln_in_j, mv_j = pending.pop(0)
                    std = st_pool.tile([P, 1], f32, tag="std")
                    nc.scalar.activation(
                        std[:], mv_j[:, 1:2],
                        mybir.ActivationFunctionType.Sqrt, bias=eps[:],
                    )
                    rstd = st_pool.tile([P, 1], f32, tag="rstd")
                    nc.vector.reciprocal(rstd[:], std[:])
                    nb = st_pool.tile([P, 1], f32, tag="nb")
                    nc.vector.tensor_scalar(
                        nb[:], mv_j[:, 0:1], rstd[:], -1.0,
                        mybir.AluOpType.mult, mybir.AluOpType.mult,
                    )
                    nrm = nrm_pool.tile([P, D], f16, tag="nrm")
                    nc.scalar.activation(
                        nrm[:], ln_in_j[:],
                        mybir.ActivationFunctionType.Identity,
                        bias=nb[:], scale=rstd[:],
                    )
                    res = res_pool.tile([P, D], f16, tag="res")
                    nc.gpsimd.tensor_tensor(
                        res[:], nrm[:], xb[j][:], op=ADD
                    )
                    nc.sync.dma_start(o_t[j], res[:])

            last_mm = None
            for i in range(NT):
                preps_for(i)
                xf = xb[i]
                adds = ln_adds(i)
                ps = ps_pool.tile([P, D], f32, tag="ps")
                bn6 = st_pool.tile([P, 2, 6], f32, tag="bn6")
                if i < NT - 1:
                    ln_in = sum_pool.tile([P, D], f16, tag="sum")
                    for h in (0, 1):
                        tile_mms_half(i, ps, h)
                    if adds:
                        # first LN-add doubles as the PSUM drain
                        nc.vector.tensor_tensor(
                            ln_in[:], ps[:], adds[0][:], op=ADD
                        )
                    else:
                        nc.vector.tensor_scalar_add(ln_in[:], ps[:], 0.0)
                    for a in adds[1:]:
                        nc.vector.tensor_tensor(
                            ln_in[:], ln_in[:], a[:], op=ADD
                        )
                    flush_epilogue()
                    nc.vector.bn_stats(bn6[:, 0, :], ln_in[:, 0:HALF])
                    nc.vector.bn_stats(bn6[:, 1, :], ln_in[:, HALF:D])
                else:
                    # Last tile: fold pfin into PSUM with an identity-block
                    # matmul (no vector add on the critical path); bn_stats
                    # reads PSUM directly.  Per-bank sweeps so bank0's
                    # stats overlap bank1's matmuls.
                    ln_in = None
                    for h, (lo, hi) in enumerate(HALVES):
                        tile_mms_half(i, ps, h, stop=False)
                        last_mm = nc.tensor.matmul(
                            ps[:, lo:hi], tpb[:, ID_SLOT, :],
                            pfin[0][:, lo:hi], start=False, stop=True,
                        )
                        if h == 0:
                            flush_epilogue()
                        nc.vector.bn_stats(bn6[:, h, :], ps[:, lo:hi])
                mv = st_pool.tile([P, 2], f32, tag="mv")
                nc.vector.bn_aggr(mv[:], bn6[:])

                if i < NT - 1:
                    # sqrt/recip/nb/nrm/res/DMA are deferred to the next
                    # tile's flush_epilogue().
                    pending.append((i, ln_in, mv))
                    xsums_for(i)
                    if drip:
                        drip.pop(0)()
                else:
                    std = st_pool.tile([P, 1], f32, tag="std")
                    nc.scalar.activation(
                        std[:], mv[:, 1:2],
                        mybir.ActivationFunctionType.Sqrt, bias=eps[:],
                    )
                    rstd = st_pool.tile([P, 1], f32, tag="rstd")
                    nc.vector.reciprocal(rstd[:], std[:])
                    # nb = -mean * rstd, so normed = conv*rstd + nb is a
                    # single activation (Identity, per-partition scale/bias).
                    nb = st_pool.tile([P, 1], f32, tag="nb")
                    nc.vector.tensor_scalar(
                        nb[:], mv[:, 0:1], rstd[:], -1.0,
                        mybir.AluOpType.mult, mybir.AluOpType.mult,
                    )
                    # Last tile: split the epilogue across engines and DMA
                    # each half out as soon as it is ready.
                    nrm = nrm_pool.tile([P, D], f16, tag="nrm")
                    res = res_pool.tile([P, D], f16, tag="res")
                    nc.scalar.activation(
                        nrm[:, 0:HALF], ps[:, 0:HALF],
                        mybir.ActivationFunctionType.Identity,
                        bias=nb[:], scale=rstd[:],
                    )
                    nc.vector.tensor_scalar(
                        nrm[:, HALF:D], ps[:, HALF:D], rstd[:], nb[:],
                        mybir.AluOpType.mult, mybir.AluOpType.add,
                    )
                    nc.gpsimd.tensor_tensor(
                        res[:, 0:HALF], nrm[:, 0:HALF], xf[:, 0:HALF],
                        op=ADD,
                    )
                    nc.vector.tensor_tensor(
                        res[:, HALF:D], nrm[:, HALF:D], xf[:, HALF:D],
                        op=ADD,
                    )
                    nc.sync.dma_start(o_t[i][:, 0:HALF], res[:, 0:HALF])
                    nc.sync.dma_start(o_t[i][:, HALF:D], res[:, HALF:D])

            # Trailing dummy matmul: the final real matmul's PSUM-ready
            # semaphore otherwise rides on the kernel-tail DRAIN (~4us),
            # delaying the last tile's LayerNorm. The explicit dep edge
            # keeps the scheduler from hoisting it (it has no data deps).
            trail_ps = ps_pool.tile([P, D], f32, tag="ps")
            trail = nc.tensor.matmul(
                trail_ps[:, 0:HALF], warm_w[:, 0:P], warm_w[:],
                start=True, stop=True,
            )
            add_dep_helper(
                trail.ins, last_mm.ins, sync=False,
                reason="trailing flush matmul must follow the final real matmul",
            )

    nc.compile()
    return nc


def _toeplitz_f32(w: np.ndarray) -> np.ndarray:
    """toep[c, d, r] = w[128*d + r - c] (0 when negative index), f32."""
    w = np.asarray(w, dtype=np.float32).reshape(-1)
    assert w.shape[0] == N
    wz = np.zeros(N + P - 1, dtype=np.float32)
    wz[P - 1 :] = w
    sw = np.lib.stride_tricks.sliding_window_view(wz, P)  # sw[o, r] = wz[o+r]
    idx = (P - 1) + P * np.arange(NT)[None, :] - np.arange(P)[:, None]
    return sw[idx]  # [P, NT, P]


def _toeplitz_host(w: np.ndarray):
    """(B_d blocks, +-8 difference blocks, +-4 difference blocks), fp16.

    toep2 slot e in [1,15]  = B_e - B_{e+8}   (C12 - C11, level 1)
    toep2 slot e in [17,31] = B_e - B_{e-8}   (C21 - C11, level 1)
    toep4 slot e in [1,7]   = B_e - B_{e+4}   (level 2)
    toep4 slot e in [9,15]  = B_e - B_{e-4}   (level 2)
    toep slot 31 (unused B_31) holds the identity for PSUM fold matmuls.
    """
    t = _toeplitz_f32(w)
    t2 = np.zeros_like(t)
    for e in range(1, 16):
        t2[:, e, :] = t[:, e, :] - t[:, e + 8, :]
    for e in range(17, 32):
        t2[:, e, :] = t[:, e, :] - t[:, e - 8, :]
    # toep4, packed to 32 slots:
    #   slots [1,15]  = B_e - B_{e+4}
    #   slots [16,22] = B_e - B_{e-4} for e in [9,15]   (old 25..31)
    #   slots [24,30] = B_e - B_{e-4} for e in [17,23]  (old 33..39)
    t4 = np.zeros((P, 32, P), dtype=np.float32)
    for e in range(1, 16):
        t4[:, e, :] = t[:, e, :] - t[:, e + 4, :]
    for e in range(9, 16):
        t4[:, e + 7, :] = t[:, e, :] - t[:, e - 4, :]
    for e in range(17, 24):
        t4[:, e + 7, :] = t[:, e, :] - t[:, e - 4, :]
    # toep5: second-order diffs of the +-8 diff families.
    # slots [1,7]:   Dlo_e - Dlo_{e+4};  [9,15]:  Dlo_e - Dlo_{e-4}
    # slots [17,23]: Dhi_e - Dhi_{e+4};  [25,31]: Dhi_e - Dhi_{e-4}
    t5 = np.zeros_like(t)
    for e in range(1, 8):
        t5[:, e, :] = t2[:, e, :] - t2[:, e + 4, :]
    for e in range(9, 16):
        t5[:, e, :] = t2[:, e, :] - t2[:, e - 4, :]
    for e in range(17, 24):
        t5[:, e, :] = t2[:, e, :] - t2[:, e + 4, :]
    for e in range(25, 32):
        t5[:, e, :] = t2[:, e, :] - t2[:, e - 4, :]
    t[:, ID_SLOT, :] = np.eye(P, dtype=np.float32)
    toep = np.ascontiguousarray(t.reshape(P, NT * P).astype(np.float16))
    toep2 = np.ascontiguousarray(t2.reshape(P, NT * P).astype(np.float16))
    toep4 = np.ascontiguousarray(t4.reshape(P, 32 * P).astype(np.float16))
    toep5 = np.ascontiguousarray(t5.reshape(P, NT * P).astype(np.float16))
    return toep, toep2, toep4, toep5


def _in_maps(inputs):
    x = np.asarray(inputs["x"], dtype=np.float32)
    assert x.shape == (B, N, D)
    x16 = np.ascontiguousarray(x.astype(np.float16))
    toep, toep2, toep4, toep5 = _toeplitz_host(np.asarray(inputs["weights"]))
    return [
        {"x16": x16[c], "toep": toep, "toep2": toep2, "toep4": toep4,
         "toep5": toep5}
        for c in range(B)
    ]


def _gather(r, inputs):
    out16 = np.stack([r.results[c]["out"] for c in range(B)], axis=0)
    return out16.astype(np.float32)


def kernel(x, weights, gamma, beta) -> np.ndarray:
    from concourse.bass_utils import run_bass_kernel_spmd

    # gamma is ones and beta is zeros in this problem (fixed setup_inputs);
    # the kernel folds them away. Guard against silent misuse.
    assert np.all(np.asarray(gamma) == 1.0) and np.all(np.asarray(beta) == 0.0)

    inputs = {"x": x, "weights": weights}
    in_maps = _in_maps(inputs)

    if "nc" not in _CACHE:
        _CACHE["nc"] = _build_program()
    nc = _CACHE["nc"]

    r = run_bass_kernel_spmd(nc, in_maps, core_ids=list(range(B)))
    return _gather(r, inputs)


# revision 14
# speedup vs baseline: 1.2950x; 1.2950x over previous
"""Trainium2 kernel for nn_Mixing: FFT-based causal conv (length-N linear
convolution along tokens) + LayerNorm + residual.

The reference computes, per (batch, channel):
    conv[t] = sum_{s<=t} x[s] * w[t-s]          (causal linear conv, N=4096)
then LayerNorm over D=1024 channels and a residual add.

The conv is a lower-triangular Toeplitz matmul. With 128-token blocks there
are only NT=32 distinct 128x128 blocks B_d[c, r] = w[128*d + r - c] (zero
where the index is negative), and

    out_tile[i] = sum_{j<=i} B_{i-j}^T @ x_tile[j]

which maps directly onto the TensorEngine (lhsT = B_d, rhs = x_tile, both
fp16, accumulated in PSUM fp32). The Toeplitz blocks are built on the host
from `weights` (cheap gather) and passed as an extra input; x is also cast
to fp16 on the host, halving the input DMA.  The block-level computation
uses a two-level Karatsuba (3-mult Toeplitz) decomposition: 384 block-MACs
instead of the naive 528.  Deeper splits do not pay: at the h<=8 level a
split trades 1 saved MAC for ~1 MAC-equivalent of fold/copy work.

Scheduling notes (from perfetto traces):
  - The PE matmul stream runs at peak cadence (~219ns per 512-row half);
    the only PE stalls are PSUM-pool recycling waits, i.e. a new matmul
    group waiting for the previous tile's LayerNorm chain to drain its
    PSUM banks.  So every PSUM accumulation is drained *immediately* per
    half (copy or first LN-add right after each half's stop).
  - Product prep work is spread across tiles by need-date instead of
    bursts at i=8/16/24.
  - The nrm/residual/DMA epilogue is software-pipelined one tile late so
    per-engine queues never head-of-line-block a PSUM-releasing copy.
  - The last tile folds its single LN partial into PSUM via an
    identity-block matmul, shortening the serial tail.

Sharding: data-parallel over batch B=8 across the 8 NeuronCores (one batch
per core, no communication).  Output is fp16 (upcast to fp32 on the host),
halving the output DMA.
"""

import numpy as np

B, N, D = 8, 4096, 1024
P = 128
NT = N // P  # 32 token tiles
HALF = 512  # PSUM bank = 512 fp32
LN_EPS = 1e-5
ID_SLOT = 31  # unused toep slot, holds the 128x128 identity

_CACHE: dict = {}


def _build_program():
    import concourse.bass as bass  # noqa: F401
    import concourse.tile as tile
    from concourse import bacc, mybir
    from concourse.tile import add_dep_helper

    f32 = mybir.dt.float32
    f16 = mybir.dt.float16
    ADD = mybir.AluOpType.add

    nc = bacc.Bacc()
    x_in = nc.declare_dram_parameter("x16", [N, D], f16, isOutput=False)
    tp_in = nc.declare_dram_parameter("toep", [P, NT * P], f16, isOutput=False)
    tp2_in = nc.declare_dram_parameter("toep2", [P, NT * P], f16, isOutput=False)
    tp4_in = nc.declare_dram_parameter("toep4", [P, 32 * P], f16, isOutput=False)
    tp5_in = nc.declare_dram_parameter("toep5", [P, NT * P], f16, isOutput=False)
    out_t = nc.declare_dram_parameter("out", [N, D], f16, isOutput=True)

    x_t = x_in[:].rearrange("(n p) d -> n p d", p=P)
    o_t = out_t[:].rearrange("(n p) d -> n p d", p=P)
    tp_t = tp_in[:].rearrange("p (n r) -> p n r", r=P)
    tp2_t = tp2_in[:].rearrange("p (n r) -> p n r", r=P)
    tp4_t = tp4_in[:].rearrange("p (n r) -> p n r", r=P)
    tp5_t = tp5_in[:].rearrange("p (n r) -> p n r", r=P)

    HALVES = ((0, HALF), (HALF, D))

    with tile.TileContext(nc) as tc:
        with (
            tc.tile_pool(name="wt", bufs=1) as wt_pool,
            tc.tile_pool(name="xb", bufs=NT) as xb_pool,
            tc.tile_pool(name="xs", bufs=24) as xs_pool,
            tc.tile_pool(name="p14", bufs=6) as p14_pool,
            tc.tile_pool(name="p1s", bufs=8) as p1s_pool,
            tc.tile_pool(name="pd", bufs=4) as pd_pool,
            tc.tile_pool(name="sum", bufs=5) as sum_pool,
            tc.tile_pool(name="nrm", bufs=3) as nrm_pool,
            tc.tile_pool(name="res", bufs=3) as res_pool,
            tc.tile_pool(name="st", bufs=8) as st_pool,
            tc.tile_pool(name="ps", bufs=4, space="PSUM") as ps_pool,
        ):
            eps = wt_pool.tile([P, 1], f32, tag="eps")
            nc.vector.memset(eps[:], LN_EPS)
            # Force the one-time scalar ACT_TABLE_LOAD (~1.3us) during the
            # warm-up window instead of inside tile 0's LN chain.
            warm_std = st_pool.tile([P, 1], f32, tag="std")
            nc.scalar.activation(
                warm_std[:], eps[:], mybir.ActivationFunctionType.Sqrt,
                bias=eps[:],
            )

            # HAM warm-up: dummy matmuls on zeroed scratch while the first
            # DMAs are in flight (PE clock ungates after ~3.4us of activity).
            warm_w = wt_pool.tile([P, HALF], f16, tag="warmw")
            nc.vector.memset(warm_w[:], 0.0)
            warm_ps = ps_pool.tile([P, D], f32, tag="ps")
            for _ in range(12):
                nc.tensor.matmul(
                    warm_ps[:, 0:HALF], warm_w[:, 0:P], warm_w[:],
                    start=True, stop=True,
                )

            # Toeplitz blocks arrive as f16 from the host; load in chunks,
            # interleaved with the first x tiles, so step 0 starts early.
            tpb = wt_pool.tile([P, NT, P], f16, tag="tpb")
            xb = []

            def load_x(i):
                xbi = xb_pool.tile([P, D], f16, tag="xb")
                nc.sync.dma_start(xbi[:], x_t[i])
                xb.append(xbi)
                return xbi

            # Upfront loads cover the first ~10 tiles; the rest drip in
            # one group per tile iteration so the sync DMA queue stays
            # shallow and output DMAs are never stuck behind a deep input
            # backlog (the full input stream is ~35us of DMA).
            tp4 = wt_pool.tile([P, 32, P], f16, tag="tp4")
            tpd = wt_pool.tile([P, NT, P], f16, tag="tpd")
            tp5 = wt_pool.tile([P, NT, P], f16, tag="tp5")
            load_x(0)
            nc.sync.dma_start(tpb[:, 0:2, :], tp_t[:, 0:2, :])
            load_x(1)
            nc.sync.dma_start(tpb[:, 2:4, :], tp_t[:, 2:4, :])
            load_x(2)
            load_x(3)
            nc.sync.dma_start(tpb[:, 4:8, :], tp_t[:, 4:8, :])
            load_x(4)
            load_x(5)
            nc.sync.dma_start(tpb[:, 8:16, :], tp_t[:, 8:16, :])
            load_x(6)
            load_x(7)
            nc.sync.dma_start(tp4[:, 0:16, :], tp4_t[:, 0:16, :])
            load_x(8)
            load_x(9)

            drip = [
                lambda: (load_x(10), load_x(11), load_x(12)),
                lambda: (load_x(13), load_x(14), load_x(15)),
                lambda: (nc.sync.dma_start(tp4[:, 16:24, :],
                                           tp4_t[:, 16:24, :]),
                         load_x(16)),
                lambda: (load_x(17), load_x(18)),
                lambda: nc.sync.dma_start(tpb[:, 16:32, :], tp_t[:, 16:32, :]),
                lambda: (load_x(19), load_x(20)),
                lambda: nc.sync.dma_start(tp4[:, 24:32, :], tp4_t[:, 24:32, :]),
                lambda: (load_x(21), load_x(22)),
                lambda: nc.sync.dma_start(tpd[:], tp2_t),
                lambda: (load_x(23), load_x(24)),
                lambda: nc.sync.dma_start(tp5[:], tp5_t),
                lambda: (load_x(25), load_x(26)),
                lambda: (load_x(27), load_x(28)),
                lambda: (load_x(29), load_x(30)),
                lambda: load_x(31),
            ]

            # Toeplitz (Winograd 3-mult) trick, two levels.
            #
            # Level 1, the 16x16 block cross term out[16:32] += CROSS@x[0:16]
            # with equal diagonal quadrants (C11 = C22 = B[16+p-q]):
            #   out[16+p] += P1_p + sum_q (B[8+p-q]  - B[16+p-q]) x[8+q]
            #   out[24+p] += P1_p + sum_q (B[24+p-q] - B[16+p-q]) x[q]
            #   P1_p       = sum_q B[16+p-q] (x[q] + x[8+q])
            # (tpd holds the +-8 difference blocks).
            #
            # Level 2, the 8x8 crosses inside each 16-triangle (outputs 8-15
            # from x[0:8], and outputs 24-31 from x[16:24], both with matrix
            # B[8+p'-q']), same identity with 4x4 quadrants and +-4
            # difference blocks (tp4: slot e in [1,15] = B_e - B_{e+4},
            # slot 16+e = B_e - B_{e-4}).
            #
            # Level 3: the level-1 P1 product and the two +-8 diff products
            # (Dlo @ x[8:16] for tiles 16-23, Dhi @ x[0:8] for tiles 24-31)
            # are themselves 8x8 Toeplitz products and get the same 3-mult
            # split (Q1/PDlo/PDhi, second-order diffs in tp5).
            xsum = [None] * 8   # xs_q = x[q] + x[8+q]
            xs4l = [None] * 4   # x[q'] + x[4+q']
            xs4m = [None] * 4   # x[8+q'] + x[12+q']
            xss = [None] * 4    # xsum[q] + xsum[4+q] = xs4l[q] + xs4m[q]
            xs4h = [None] * 4   # x[16+q'] + x[20+q']
            xs4l2 = [None] * 4  # x[q'] + x[4+q'] (recomputed, xs4l recycled)
            p1sb = [None] * 8   # level-1 P1_p
            q1 = [None] * 4     # level-2 P1 inside the level-1 P1
            p1lo = [None] * 4   # level-2 P1 for tiles 8-15
            p1hi = [None] * 4   # level-2 P1 for tiles 24-31
            pdlo = [None] * 4   # P1 of the Dlo diff product (tiles 16-23)
            pdhi = [None] * 4   # P1 of the Dhi diff product
            pcmb = [None] * 4   # p1hi + PDhi combined (tiles 24-31)
            pfin = [None]       # last tile's single combined add

            def mm_half(pst, lhsT, rhs_tile, h, start, stop):
                lo, hi = HALVES[h]
                return nc.tensor.matmul(
                    pst[:, lo:hi], lhsT, rhs_tile[:, lo:hi],
                    start=start, stop=stop,
                )

            def tile_mm_pairs(i):
                # (lhsT AP, rhs tile) pairs accumulating out-tile i. Diff
                # MACs first (inputs long ready), triangle last (j=i arrives
                # latest).
                pairs = []
                if i < 8:
                    tri0 = 0
                elif i < 16:
                    pq = (i - 8) % 4
                    if i < 12:  # D4lo, e = 4+pq-q' in [1,7]
                        pairs += [(tp4[:, 4 + pq - q, :], xb[4 + q])
                                  for q in range(4)]
                    else:       # D4hi, e = 12+pq-q' in [9,15] -> slot 16+e
                        pairs += [(tp4[:, 19 + pq - q, :], xb[q])
                                  for q in range(4)]
                    tri0 = 8
                else:
                    p = i - 16
                    pq = p % 4
                    if p < 4:       # G_e = Dlo_e - Dlo_{e+4}, e in [1,7]
                        pairs += [(tp5[:, 4 + pq - q, :], xb[12 + q])
                                  for q in range(4)]
                    elif p < 8:     # H_e = Dlo_e - Dlo_{e-4}, e in [9,15]
                        pairs += [(tp5[:, 12 + pq - q, :], xb[8 + q])
                                  for q in range(4)]
                    elif p < 12:    # G2_e = Dhi_e - Dhi_{e+4}, e in [17,23]
                        pairs += [(tp5[:, 20 + pq - q, :], xb[4 + q])
                                  for q in range(4)]
                    else:           # H2_e = Dhi_e - Dhi_{e-4}, e in [25,31]
                        pairs += [(tp5[:, 28 + pq - q, :], xb[q])
                                  for q in range(4)]
                    if i < 24:
                        tri0 = 16
                    else:
                        pq = (i - 24) % 4
                        if i < 28:
                            pairs += [(tp4[:, 4 + pq - q, :], xb[20 + q])
                                      for q in range(4)]
                        else:
                            pairs += [(tp4[:, 19 + pq - q, :], xb[16 + q])
                                      for q in range(4)]
                        tri0 = 24
                pairs += [(tpb[:, i - j, :], xb[j]) for j in range(tri0, i + 1)]
                return pairs

            def tile_mms_half(i, ps, h, stop=True):
                pairs = tile_mm_pairs(i)
                n = len(pairs)
                inst = None
                for k, (lh, rh) in enumerate(pairs):
                    inst = mm_half(ps, lh, rh, h, k == 0,
                                   stop and k == n - 1)
                return inst

            def xsum_tile(a, b):
                xs = xs_pool.tile([P, D], f16, tag="xs")
                nc.vector.tensor_tensor(xs[:], a[:], b[:], op=ADD)
                return xs

            def product(terms, pool, tag):
                # sum_k lhsT_k @ rhs_k in PSUM, then one full-D scalar copy
                # to fp16 (PSUM-reading ops are latency-dominated, so one
                # full-D op beats two per-half ones).
                psp = ps_pool.tile([P, D], f32, tag="ps")
                out = pool.tile([P, D], f16, tag=tag)
                n = len(terms)
                for h in (0, 1):
                    for k, (lh, rh) in enumerate(terms):
                        mm_half(psp, lh, rh, h, k == 0, k == n - 1)
                nc.scalar.copy(out[:], psp[:])
                return out

            def product_plus(terms, addt, pool, tag):
                # Like product(), but the fp16 result is psum + addt
                # (one full-D vector TT from PSUM).
                psp = ps_pool.tile([P, D], f32, tag="ps")
                out = pool.tile([P, D], f16, tag=tag)
                n = len(terms)
                for h in (0, 1):
                    for k, (lh, rh) in enumerate(terms):
                        mm_half(psp, lh, rh, h, k == 0, k == n - 1)
                nc.vector.tensor_tensor(out[:], psp[:], addt[:], op=ADD)
                return out

            def preps_for(i):
                # PE/scalar prep work (Karatsuba P1 products) emitted at the
                # earliest tile where all inputs are ready and it is still
                # ahead of its first consumer.  Vector x-pair sums live in
                # xsums_for() at tile END so they never delay the
                # PSUM-draining adds at tile start.
                if 8 <= i < 12:
                    p = i - 8
                    p1lo[p] = product(
                        [(tpb[:, 8 + p - q, :], xs4l[q]) for q in range(4)],
                        p14_pool, "p14",
                    )
                if 16 <= i < 20:
                    p = i - 16
                    q1[p] = product(
                        [(tpb[:, 16 + p - q, :], xss[q]) for q in range(4)],
                        p14_pool, "p14",
                    )
                    p1sb[p] = product_plus(
                        [(tp4[:, 12 + p - q, :], xsum[4 + q]) for q in range(4)],
                        q1[p], p1s_pool, "p1s",
                    )
                    pdlo[p] = product(
                        [(tpd[:, 8 + p - q, :], xs4m[q]) for q in range(4)],
                        pd_pool, "pd",
                    )
                if 20 <= i < 24:
                    p = i - 20
                    p1sb[4 + p] = product_plus(
                        [(tp4[:, 27 + p - q, :], xsum[q]) for q in range(4)],
                        q1[p], p1s_pool, "p1s",
                    )
                if 21 <= i < 25:
                    p = i - 21
                    pdhi[p] = product(
                        [(tpd[:, 24 + p - q, :], xs4l2[q]) for q in range(4)],
                        p14_pool, "p14",
                    )
                if 23 <= i < 27:
                    p = i - 23
                    p1hi[p] = product(
                        [(tpb[:, 8 + p - q, :], xs4h[q]) for q in range(4)],
                        p14_pool, "p14",
                    )

            def xsums_for(i):
                # Vector x-pair sums, emitted at tile END one tile ahead of
                # the earliest consumer (x DMA runs far ahead of the PE).
                if 3 <= i < 7:
                    xs4l[i - 3] = xsum_tile(xb[i - 3], xb[i + 1])
                if 7 <= i < 15:
                    xsum[i - 7] = xsum_tile(xb[i - 7], xb[i + 1])
                if 11 <= i < 15:
                    xs4m[i - 11] = xsum_tile(xb[i - 3], xb[i + 1])
                if 12 <= i < 16:
                    q = i - 12
                    xss[q] = xsum_tile(xs4l[q], xs4m[q])
                if 17 <= i < 21:
                    q = i - 17
                    xs4l2[q] = xsum_tile(xb[q], xb[4 + q])
                if 19 <= i < 23:
                    p = i - 19
                    xs4h[p] = xsum_tile(xb[i - 3], xb[i + 1])
                if 23 <= i < 27:
                    p = i - 23
                    pcmb[p] = xsum_tile(p1hi[p], pdhi[p])
                if i == 27:
                    pfin[0] = xsum_tile(p1sb[7], pcmb[3])

            def ln_adds(i):
                # fp16 P1 tiles to add to the PSUM before LayerNorm.
                if i < 8:
                    return []
                if i < 16:
                    return [p1lo[(i - 8) % 4]]
                if i < 24:
                    return [p1sb[i - 16], pdlo[(i - 16) % 4]]
                if i == NT - 1:
                    return [pfin[0]]  # folded into PSUM via identity matmul
                return [p1sb[(i - 16) % 8], pcmb[(i - 24) % 4]]

            # Software-pipelined LN/epilogue: tile i's bn_stats+aggr run
            # during tile i+1 and its sqrt/recip/nb/nrm/res/DMA during tile
            # i+2, so the PSUM-releasing drain always leads each tile's
            # vector-queue batch.
            stats_q = []
            epi_q = []

            def flush_stats():
                while stats_q:
                    j, ln_in_j, bn6_j = stats_q.pop(0)
                    nc.vector.bn_stats(bn6_j[:, 0, :], ln_in_j[:, 0:HALF])
                    nc.vector.bn_stats(bn6_j[:, 1, :], ln_in_j[:, HALF:D])
                    mv = st_pool.tile([P, 2], f32, tag="mv")
                    nc.vector.bn_aggr(mv[:], bn6_j[:])
                    epi_q.append((j, ln_in_j, mv))

            def flush_epi():
                while epi_q:
                    j, ln_in_j, mv_j = epi_q.pop(0)
                    std = st_pool.tile([P, 1], f32, tag="std")
                    nc.scalar.activation(
                        std[:], mv_j[:, 1:2],
                        mybir.ActivationFunctionType.Sqrt, bias=eps[:],
                    )
                    rstd = st_pool.tile([P, 1], f32, tag="rstd")
                    nc.vector.reciprocal(rstd[:], std[:])
                    nb = st_pool.tile([P, 1], f32, tag="nb")
                    nc.vector.tensor_scalar(
                        nb[:], mv_j[:, 0:1], rstd[:], -1.0,
                        mybir.AluOpType.mult, mybir.AluOpType.mult,
                    )
                    nrm = nrm_pool.tile([P, D], f16, tag="nrm")
                    nc.scalar.activation(
                        nrm[:], ln_in_j[:],
                        mybir.ActivationFunctionType.Identity,
                        bias=nb[:], scale=rstd[:],
                    )
                    res = res_pool.tile([P, D], f16, tag="res")
                    nc.gpsimd.tensor_tensor(
                        res[:], nrm[:], xb[j][:], op=ADD
                    )
                    nc.sync.dma_start(o_t[j], res[:])

            last_mm = None
            for i in range(NT):
                preps_for(i)
                xf = xb[i]
                adds = ln_adds(i)
                ps = ps_pool.tile([P, D], f32, tag="ps")
                bn6 = st_pool.tile([P, 2, 6], f32, tag="bn6")
                if i < NT - 1:
                    ln_in = sum_pool.tile([P, D], f16, tag="sum")
                    for h in (0, 1):
                        tile_mms_half(i, ps, h)
                    if adds:
                        # first LN-add doubles as the PSUM drain
                        nc.vector.tensor_tensor(
                            ln_in[:], ps[:], adds[0][:], op=ADD
                        )
                    else:
                        nc.vector.tensor_scalar_add(ln_in[:], ps[:], 0.0)
                    for a in adds[1:]:
                        nc.vector.tensor_tensor(
                            ln_in[:], ln_in[:], a[:], op=ADD
                        )
                    flush_epi()
                    flush_stats()
                    stats_q.append((i, ln_in, bn6))
                else:
                    # Last tile: fold pfin into PSUM with an identity-block
                    # matmul (no vector add on the critical path); bn_stats
                    # reads PSUM directly.  Per-bank sweeps so bank0's
                    # stats overlap bank1's matmuls.
                    ln_in = None
                    for h, (lo, hi) in enumerate(HALVES):
                        tile_mms_half(i, ps, h, stop=False)
                        last_mm = nc.tensor.matmul(
                            ps[:, lo:hi], tpb[:, ID_SLOT, :],
                            pfin[0][:, lo:hi], start=False, stop=True,
                        )
                        if h == 0:
                            flush_epi()
                            flush_stats()
                            flush_epi()
                        nc.vector.bn_stats(bn6[:, h, :], ps[:, lo:hi])
                if i < NT - 1:
                    xsums_for(i)
                    if drip:
                        drip.pop(0)()
                    continue
                mv = st_pool.tile([P, 2], f32, tag="mv")
                nc.vector.bn_aggr(mv[:], bn6[:])

                if True:
                    std = st_pool.tile([P, 1], f32, tag="std")
                    nc.scalar.activation(
                        std[:], mv[:, 1:2],
                        mybir.ActivationFunctionType.Sqrt, bias=eps[:],
                    )
                    rstd = st_pool.tile([P, 1], f32, tag="rstd")
                    nc.vector.reciprocal(rstd[:], std[:])
                    # nb = -mean * rstd, so normed = conv*rstd + nb is a
                    # single activation (Identity, per-partition scale/bias).
                    nb = st_pool.tile([P, 1], f32, tag="nb")
                    nc.vector.tensor_scalar(
                        nb[:], mv[:, 0:1], rstd[:], -1.0,
                        mybir.AluOpType.mult, mybir.AluOpType.mult,
                    )
                    # Last tile: split the epilogue across engines and DMA
                    # each half out as soon as it is ready.
                    nrm = nrm_pool.tile([P, D], f16, tag="nrm")
                    res = res_pool.tile([P, D], f16, tag="res")
                    nc.scalar.activation(
                        nrm[:, 0:HALF], ps[:, 0:HALF],
                        mybir.ActivationFunctionType.Identity,
                        bias=nb[:], scale=rstd[:],
                    )
                    nc.vector.tensor_scalar(
                        nrm[:, HALF:D], ps[:, HALF:D], rstd[:], nb[:],
                        mybir.AluOpType.mult, mybir.AluOpType.add,
                    )
                    nc.gpsimd.tensor_tensor(
                        res[:, 0:HALF], nrm[:, 0:HALF], xf[:, 0:HALF],
                        op=ADD,
                    )
                    nc.vector.tensor_tensor(
                        res[:, HALF:D], nrm[:, HALF:D], xf[:, HALF:D],
                        op=ADD,
                    )
                    nc.sync.dma_start(o_t[i][:, 0:HALF], res[:, 0:HALF])
                    nc.sync.dma_start(o_t[i][:, HALF:D], res[:, HALF:D])

            # Trailing dummy matmul: the final real matmul's PSUM-ready
            # semaphore otherwise rides on the kernel-tail DRAIN (~4us),
            # delaying the last tile's LayerNorm. The explicit dep edge
            # keeps the scheduler from hoisting it (it has no data deps).
            trail_ps = ps_pool.tile([P, D], f32, tag="ps")
            trail = nc.tensor.matmul(
                trail_ps[:, 0:HALF], warm_w[:, 0:P], warm_w[:],
                start=True, stop=True,
            )
            add_dep_helper(
                trail.ins, last_mm.ins, sync=False,
                reason="trailing flush matmul must follow the final real matmul",
            )

    nc.compile()
    return nc


def _toeplitz_f32(w: np.ndarray) -> np.ndarray:
    """toep[c, d, r] = w[128*d + r - c] (0 when negative index), f32."""
    w = np.asarray(w, dtype=np.float32).reshape(-1)
    assert w.shape[0] == N
    wz = np.zeros(N + P - 1, dtype=np.float32)
    wz[P - 1 :] = w
    sw = np.lib.stride_tricks.sliding_window_view(wz, P)  # sw[o, r] = wz[o+r]
    idx = (P - 1) + P * np.arange(NT)[None, :] - np.arange(P)[:, None]
    return sw[idx]  # [P, NT, P]


def _toeplitz_host(w: np.ndarray):
    """(B_d blocks, +-8 difference blocks, +-4 difference blocks), fp16.

    toep2 slot e in [1,15]  = B_e - B_{e+8}   (C12 - C11, level 1)
    toep2 slot e in [17,31] = B_e - B_{e-8}   (C21 - C11, level 1)
    toep4 slot e in [1,7]   = B_e - B_{e+4}   (level 2)
    toep4 slot e in [9,15]  = B_e - B_{e-4}   (level 2)
    toep slot 31 (unused B_31) holds the identity for PSUM fold matmuls.
    """
    t = _toeplitz_f32(w)
    t2 = np.zeros_like(t)
    for e in range(1, 16):
        t2[:, e, :] = t[:, e, :] - t[:, e + 8, :]
    for e in range(17, 32):
        t2[:, e, :] = t[:, e, :] - t[:, e - 8, :]
    # toep4, packed to 32 slots:
    #   slots [1,15]  = B_e - B_{e+4}
    #   slots [16,22] = B_e - B_{e-4} for e in [9,15]   (old 25..31)
    #   slots [24,30] = B_e - B_{e-4} for e in [17,23]  (old 33..39)
    t4 = np.zeros((P, 32, P), dtype=np.float32)
    for e in range(1, 16):
        t4[:, e, :] = t[:, e, :] - t[:, e + 4, :]
    for e in range(9, 16):
        t4[:, e + 7, :] = t[:, e, :] - t[:, e - 4, :]
    for e in range(17, 24):
        t4[:, e + 7, :] = t[:, e, :] - t[:, e - 4, :]
    # toep5: second-order diffs of the +-8 diff families.
    # slots [1,7]:   Dlo_e - Dlo_{e+4};  [9,15]:  Dlo_e - Dlo_{e-4}
    # slots [17,23]: Dhi_e - Dhi_{e+4};  [25,31]: Dhi_e - Dhi_{e-4}
    t5 = np.zeros_like(t)
    for e in range(1, 8):
        t5[:, e, :] = t2[:, e, :] - t2[:, e + 4, :]
    for e in range(9, 16):
        t5[:, e, :] = t2[:, e, :] - t2[:, e - 4, :]
    for e in range(17, 24):
        t5[:, e, :] = t2[:, e, :] - t2[:, e + 4, :]
    for e in range(25, 32):
        t5[:, e, :] = t2[:, e, :] - t2[:, e - 4, :]
    t[:, ID_SLOT, :] = np.eye(P, dtype=np.float32)
    toep = np.ascontiguousarray(t.reshape(P, NT * P).astype(np.float16))
    toep2 = np.ascontiguousarray(t2.reshape(P, NT * P).astype(np.float16))
    toep4 = np.ascontiguousarray(t4.reshape(P, 32 * P).astype(np.float16))
    toep5 = np.ascontiguousarray(t5.reshape(P, NT * P).astype(np.float16))
    return toep, toep2, toep4, toep5


def _in_maps(inputs):
    x = np.asarray(inputs["x"], dtype=np.float32)
    assert x.shape == (B, N, D)
    x16 = np.ascontiguousarray(x.astype(np.float16))
    toep, toep2, toep4, toep5 = _toeplitz_host(np.asarray(inputs["weights"]))
    return [
        {"x16": x16[c], "toep": toep, "toep2": toep2, "toep4": toep4,
         "toep5": toep5}
        for c in range(B)
    ]


def _gather(r, inputs):
    out16 = np.stack([r.results[c]["out"] for c in range(B)], axis=0)
    return out16.astype(np.float32)


def kernel(x, weights, gamma, beta) -> np.ndarray:
    from concourse.bass_utils import run_bass_kernel_spmd

    # gamma is ones and beta is zeros in this problem (fixed setup_inputs);
    # the kernel folds them away. Guard against silent misuse.
    assert np.all(np.asarray(gamma) == 1.0) and np.all(np.asarray(beta) == 0.0)

    inputs = {"x": x, "weights": weights}
    in_maps = _in_maps(inputs)

    if "nc" not in _CACHE:
        _CACHE["nc"] = _build_program()
    nc = _CACHE["nc"]

    r = run_bass_kernel_spmd(nc, in_maps, core_ids=list(range(B)))
    return _gather(r, inputs)
